# revision 4
# baseline (speedup 1.0000x reference)
"""3x3 median blur (zero padding) on (16, 3, 512, 512) f32 for 8 NeuronCores.

Data-parallel over batch: 2 images x 3 channels = 6 image slabs per core.
Per core, a 15 op/px min-max median network runs on DVE (bf16, 2x packed
mode), exploiting pair-sharing in BOTH directions:
  - vertically, adjacent output rows share the pair compare-exchange;
  - horizontally, the image is de-interleaved into even/odd column
    planes so adjacent output columns share their pair CE while keeping
    every operand last-dim-contiguous (preserves the packed DVE mode).
ACT does the f32->bf16 cast into plane layout; Pool zeroes halo rows and
pads.  The output is stored in plane layout (even cols then odd cols per
image) and re-interleaved on the host at zero hardware cost.  Loads,
casts, halo-shift DMAs, finals and stores are split per image / row-half
/ plane for pipelining; stores are scheduler-prioritized to stream out
during compute.
"""

import numpy as np

B, C, H, W = 16, 3, 512, 512
N_CORES = 8
B_LOC = B // N_CORES          # 2 batches per core
IMGS = B_LOC * C              # 6 images per core
G = 2                         # images per instruction block
NBLK = IMGS // G              # 3 blocks
R = 4                         # output rows per partition (128*4 = 512)
WP = W + 2                    # padded row width (f32 load tile)
WH = W // 2                   # 256 plane columns
TBW = WH + 2                  # tb plane width incl zero pad cols (258)
XSPLIT = 512                  # single DVE region (Pool cannot run min/max)

_STATE = {}


def _mk_ap(base_ap, offset, pattern):
    """Clone an AP with a manual [step, count] pattern (element units)."""
    import concourse.mybir as mybir

    ap = base_ap.copy()
    ap.ap = mybir.VecI64Pair(pattern)
    ap.offset = offset
    return ap


def _build_nc():
    import os as _os
    import concourse.bacc as bacc
    import concourse.mybir as mybir
    from concourse.tile import TileContext

    f32 = mybir.dt.float32
    bf16 = mybir.dt.bfloat16
    Alu = mybir.AluOpType

    nc = bacc.Bacc("TRN2")
    x = nc.dram_tensor("x", [IMGS, H, W], f32, kind="ExternalInput")
    y = nc.dram_tensor("y", [IMGS, H, W], bf16, kind="ExternalOutput")

    with TileContext(nc) as tc:
        with (
            tc.tile_pool(name="inp", bufs=3) as inp,
            tc.tile_pool(name="castp", bufs=3) as castp,
            tc.tile_pool(name="midv", bufs=1) as midv,
            tc.tile_pool(name="midp", bufs=1) as midp,
            tc.tile_pool(name="outp", bufs=2) as outp,
        ):
            def phase1(E, mid, tag, tb, X0, OW):
                """Vertical sort3 for plane cols k in [K0-1, K0+KW+1) on
                engine E (covers out cols [X0,X0+OW) + halo cols)."""
                TT = lambda out, in0, in1, op: E.tensor_tensor(
                    out=out, in0=in0, in1=in1, op=op
                )
                K0, KW = X0 // 2, OW // 2
                kw = KW + 2
                pvmin = mid.tile([128, G, 2, 2, kw], bf16, tag=f"pvmin{tag}")
                pvmax = mid.tile([128, G, 2, 2, kw], bf16, tag=f"pvmax{tag}")
                s0 = mid.tile([128, G, R, 2, kw], bf16, tag=f"s0{tag}")
                s1 = mid.tile([128, G, R, 2, kw], bf16, tag=f"s1{tag}")
                s2 = mid.tile([128, G, R, 2, kw], bf16, tag=f"s2{tag}")

                def c_ap(g, gg):
                    return _mk_ap(
                        tb[:],
                        (g * 6 + 3 * gg) * 2 * TBW + K0,
                        [[G * 6 * 2 * TBW, 128], [2 * 2 * TBW, 2], [TBW, 2], [1, kw]],
                    )

                def pv_ap(tile, g):
                    return _mk_ap(
                        tile[:], g * 2 * 2 * kw,
                        [[G * 2 * 2 * kw, 128], [2 * kw, 2], [kw, 2], [1, kw]],
                    )

                def s_ap(tile, g, gg):
                    return _mk_ap(
                        tile[:],
                        (g * R + gg) * 2 * kw,
                        [[G * R * 2 * kw, 128], [2 * 2 * kw, 2], [kw, 2], [1, kw]],
                    )

                for g in range(G):
                    ca = tb[:, g, 1:5:2, :, K0 : K0 + kw]
                    cb = tb[:, g, 2:6:2, :, K0 : K0 + kw]
                    TT(pvmax[:, g], ca, cb, Alu.max)
                    TT(pvmin[:, g], ca, cb, Alu.min)
                    for gg in range(2):
                        TT(s_ap(s2, g, gg), pv_ap(pvmax, g), c_ap(g, gg), Alu.max)
                        TT(s_ap(s0, g, gg), pv_ap(pvmin, g), c_ap(g, gg), Alu.min)
                        TT(s_ap(s1, g, gg), pv_ap(pvmax, g), c_ap(g, gg), Alu.min)
                        TT(s_ap(s1, g, gg), pv_ap(pvmin, g), s_ap(s1, g, gg), Alu.max)
                return s0, s1, s2

            def phase2(E, mid, tag, s0, s1, s2, yv, img0, X0, OW):
                """Horizontal merge in plane layout + merged per-image store."""
                TT = lambda out, in0, in1, op: E.tensor_tensor(
                    out=out, in0=in0, in1=in1, op=op
                )
                K0, KW = X0 // 2, OW // 2

                def EP(s):   # E plane, pair cols   k
                    return s[:, :, :, 0, 1 : 1 + KW]
                def OP(s):   # O plane, pair cols   k
                    return s[:, :, :, 1, 1 : 1 + KW]
                def OL(s):   # O plane, cols  k-1
                    return s[:, :, :, 1, 0:KW]
                def ER(s):   # E plane, cols  k+1
                    return s[:, :, :, 0, 2 : 2 + KW]

                pM = mid.tile([128, G, R, KW], bf16, tag=f"pM{tag}")
                A = mid.tile([128, G, R, 2, KW], bf16, tag=f"A{tag}")
                pm2 = mid.tile([128, G, R, KW], bf16, tag=f"pm2{tag}")
                Cc = mid.tile([128, G, R, 2, KW], bf16, tag=f"Cc{tag}")
                # A = max3(s0) per plane
                TT(pM[:], EP(s0), OP(s0), Alu.max)
                TT(A[:, :, :, 0, :], pM[:], OL(s0), Alu.max)
                TT(A[:, :, :, 1, :], pM[:], ER(s0), Alu.max)
                # Cc = min3(s2) per plane
                TT(pm2[:], EP(s2), OP(s2), Alu.min)
                TT(Cc[:, :, :, 0, :], pm2[:], OL(s2), Alu.min)
                TT(Cc[:, :, :, 1, :], pm2[:], ER(s2), Alu.min)
                # B = med3(s1) per plane
                pbm = mid.tile([128, G, R, KW], bf16, tag=f"pbm{tag}")
                pbM = mid.tile([128, G, R, KW], bf16, tag=f"pbM{tag}")
                tB = mid.tile([128, G, R, 2, KW], bf16, tag=f"tB{tag}")
                TT(pbm[:], EP(s1), OP(s1), Alu.min)
                TT(pbM[:], EP(s1), OP(s1), Alu.max)
                TT(tB[:, :, :, 0, :], pbM[:], OL(s1), Alu.min)
                TT(tB[:, :, :, 0, :], pbm[:], tB[:, :, :, 0, :], Alu.max)
                TT(tB[:, :, :, 1, :], pbM[:], ER(s1), Alu.min)
                TT(tB[:, :, :, 1, :], pbm[:], tB[:, :, :, 1, :], Alu.max)
                # final med3(A, B, Cc) per image g (planes folded, contiguous)
                u = mid.tile([128, G, R, 2, KW], bf16, tag=f"u{tag}")
                out_t = outp.tile([128, G, R, 2, KW], bf16, tag=f"out{tag}")
                for g in range(G):
                    for pl in range(2):
                        TT(u[:, g, :, pl], A[:, g, :, pl], tB[:, g, :, pl], Alu.min)
                        TT(A[:, g, :, pl], A[:, g, :, pl], tB[:, g, :, pl], Alu.max)
                        TT(Cc[:, g, :, pl], A[:, g, :, pl], Cc[:, g, :, pl], Alu.min)
                        TT(out_t[:, g, :, pl], u[:, g, :, pl], Cc[:, g, :, pl], Alu.max)
                        # per-plane store (host decodes the plane layout)
                        with tc.high_priority(offset=70):
                            yr = yv[img0 + g].rearrange("(p r) w -> p r w", p=128)
                            nc.sync.dma_start(
                                out=yr[:, :, 2 * K0 + pl * KW : 2 * K0 + (pl + 1) * KW],
                                in_=out_t[:, g, :, pl],
                            )

            # tiny ACT warmup so LoadActFuncSet runs at t=0, off the
            # critical path of the first cast
            warm = castp.tile([128, 2], bf16, tag="warm")
            nc.scalar.memzero(warm[:])

            for _rep in range(int(_os.environ.get("KREPS", "1"))):
              for blk in range(NBLK):
                img0 = blk * G

                # ---- load f32 central rows (per image):
                # t[p, g, r, 1+w] = x[img0+g, 4p+r, w]
                t = inp.tile([128, G, R, WP], f32, tag="t_in")
                for g in range(G):
                    xr = x[img0 + g].rearrange("(p r) w -> p r w", p=128)
                    nc.sync.dma_start(out=t[:, g, 0:2, 1 : W + 1], in_=xr[:, 0:2, :])
                    nc.sync.dma_start(out=t[:, g, 2:4, 1 : W + 1], in_=xr[:, 2:4, :])

                # ---- bf16 plane tile with halo rows:
                # tb[p, g, j, pl, 1+k] = x[img0+g, 4p+j-1, 2k+pl], j=1..4
                tb = castp.tile([128, G, 6, 2, TBW], bf16, tag="tb")
                # zero halo rows 0 and 5 (all partitions, both planes) (Pool,
                # keeping ACT free to start casting as soon as loads land)
                nc.gpsimd.memset(tb[:, :, 0:6:5, :, :], 0.0)
                for g in range(G):
                    # zero pad cols 0,257 of rows 1..4 (tiny, Pool)
                    nc.gpsimd.memset(tb[:, g, 1:5, :, 0 : TBW : TBW - 1], 0.0)
                    # cast central rows of image g into planes (ACT)
                    for rh in range(2):
                        nc.scalar.copy(
                            out=tb[:, g, 1 + 2 * rh : 3 + 2 * rh, 0, 1 : 1 + WH],
                            in_=t[:, g, 2 * rh : 2 * rh + 2, 1 : W + 1 : 2],
                        )
                        nc.scalar.copy(
                            out=tb[:, g, 1 + 2 * rh : 3 + 2 * rh, 1, 1 : 1 + WH],
                            in_=t[:, g, 2 * rh : 2 * rh + 2, 2 : W + 2 : 2],
                        )
                    # halo above: tb row 0 of partition p = tb row 4 of p-1
                    nc.sync.dma_start(
                        out=tb[1:128, g, 0, :, :], in_=tb[0:127, g, 4, :, :]
                    )
                    # halo below: tb row 5 of partition p = tb row 1 of p+1
                    nc.sync.dma_start(
                        out=tb[0:127, g, 5, :, :], in_=tb[1:128, g, 1, :, :]
                    )

                # ---- the network, all on DVE (neuronxcc rejects Pool min/max)
                sv = phase1(nc.vector, midv, "v", tb, 0, XSPLIT)
                phase2(nc.vector, midv, "v", *sv, y, img0, 0, XSPLIT)
    nc.compile()
    return nc


def _get_nc():
    if "nc" not in _STATE:
        _STATE["nc"] = _build_nc()
    return _STATE["nc"]


def _decode(yarr):
    """Undo the plane-blocked store layout -> real column order."""
    KV = XSPLIT // 2
    KP = (W - XSPLIT) // 2
    out = np.empty_like(yarr)
    out[..., 0:XSPLIT:2] = yarr[..., 0:KV]
    out[..., 1:XSPLIT:2] = yarr[..., KV : 2 * KV]
    out[..., XSPLIT::2] = yarr[..., 2 * KV : 2 * KV + KP]
    out[..., XSPLIT + 1 :: 2] = yarr[..., 2 * KV + KP :]
    return out


def kernel(x: np.ndarray) -> np.ndarray:
    from concourse.bass_utils import run_bass_kernel_spmd

    x = np.ascontiguousarray(np.asarray(x, dtype=np.float32))
    assert x.shape == (B, C, H, W), x.shape

    nc = _get_nc()
    in_maps = [
        {"x": x[i * B_LOC : (i + 1) * B_LOC].reshape(IMGS, H, W)}
        for i in range(N_CORES)
    ]
    res = run_bass_kernel_spmd(nc, in_maps, core_ids=list(range(N_CORES)))
    _STATE["last_results"] = res
    out = np.empty((B, C, H, W), dtype=np.float32)
    ov = out.reshape(N_CORES, IMGS, H, W)
    for i, r in enumerate(res.results):
        ov[i] = _decode(np.asarray(r["y"]).astype(np.float32))
    return out


# revision 5
# speedup vs baseline: 1.8826x; 1.8826x over previous
"""3x3 median blur (zero padding) on (16, 3, 512, 512) f32 for 8 NeuronCores.

Data-parallel over batch: 2 images x 3 channels = 6 image slabs per core.
Per core, a 15 op/px min-max median network runs on DVE (bf16, 2x packed
mode), exploiting pair-sharing in BOTH directions:
  - vertically, adjacent output rows share the pair compare-exchange;
  - horizontally, the image is de-interleaved into even/odd column
    planes so adjacent output columns share their pair CE while keeping
    every operand last-dim-contiguous (preserves the packed DVE mode).
ACT does the f32->bf16 cast into plane layout; Pool zeroes halo rows and
pads.  The output is stored in plane layout (even cols then odd cols per
image) and re-interleaved on the host at zero hardware cost.

Pipelining: loads/casts split per image and row-half so DVE starts right
after the first image lands; stores are merged and scheduler-prioritized
to stream during compute; the steady-state blocks use image-merged
vertical ops and whole-block finals (fewest instructions), while block 0
and the last block stay fine-grained to shrink startup and tail.
"""

import numpy as np

B, C, H, W = 16, 3, 512, 512
N_CORES = 8
B_LOC = B // N_CORES          # 2 batches per core
IMGS = B_LOC * C              # 6 images per core
G = 2                         # images per instruction block
NBLK = IMGS // G              # 3 blocks
R = 4                         # output rows per partition (128*4 = 512)
WP = W + 2                    # padded row width (f32 load tile)
WH = W // 2                   # 256 plane columns
TBW = WH + 2                  # tb plane width incl zero pad cols (258)
XSPLIT = 512                  # single DVE region (Pool cannot run min/max)

_STATE = {}


def _mk_ap(base_ap, offset, pattern):
    """Clone an AP with a manual [step, count] pattern (element units)."""
    import concourse.mybir as mybir

    ap = base_ap.copy()
    ap.ap = mybir.VecI64Pair(pattern)
    ap.offset = offset
    return ap


def _build_nc():
    import os as _os
    import concourse.bacc as bacc
    import concourse.mybir as mybir
    from concourse.tile import TileContext

    f32 = mybir.dt.float32
    bf16 = mybir.dt.bfloat16
    Alu = mybir.AluOpType

    nc = bacc.Bacc("TRN2")
    x = nc.dram_tensor("x", [IMGS, H, W], f32, kind="ExternalInput")
    y = nc.dram_tensor("y", [IMGS, H, W], bf16, kind="ExternalOutput")

    with TileContext(nc) as tc:
        with (
            tc.tile_pool(name="inp", bufs=3) as inp,
            tc.tile_pool(name="castp", bufs=3) as castp,
            tc.tile_pool(name="midv", bufs=1) as midv,
            tc.tile_pool(name="midp", bufs=1) as midp,
            tc.tile_pool(name="outp", bufs=2) as outp,
        ):
            def phase1(E, mid, tag, tb, X0, OW, blk):
                """Vertical sort3 for plane cols k in [K0-1, K0+KW+1) on
                engine E (covers out cols [X0,X0+OW) + halo cols)."""
                TT = lambda out, in0, in1, op: E.tensor_tensor(
                    out=out, in0=in0, in1=in1, op=op
                )
                K0, KW = X0 // 2, OW // 2
                kw = KW + 2
                pvmin = mid.tile([128, G, 2, 2, kw], bf16, tag=f"pvmin{tag}")
                pvmax = mid.tile([128, G, 2, 2, kw], bf16, tag=f"pvmax{tag}")
                s0 = mid.tile([128, G, R, 2, kw], bf16, tag=f"s0{tag}")
                s1 = mid.tile([128, G, R, 2, kw], bf16, tag=f"s1{tag}")
                s2 = mid.tile([128, G, R, 2, kw], bf16, tag=f"s2{tag}")

                def c_ap(g, gg):
                    return _mk_ap(
                        tb[:],
                        (g * 6 + 3 * gg) * 2 * TBW + K0,
                        [[G * 6 * 2 * TBW, 128], [2 * 2 * TBW, 2], [TBW, 2], [1, kw]],
                    )

                def pv_ap(tile, g):
                    return _mk_ap(
                        tile[:], g * 2 * 2 * kw,
                        [[G * 2 * 2 * kw, 128], [2 * kw, 2], [kw, 2], [1, kw]],
                    )

                def s_ap(tile, g, gg):
                    return _mk_ap(
                        tile[:],
                        (g * R + gg) * 2 * kw,
                        [[G * R * 2 * kw, 128], [2 * 2 * kw, 2], [kw, 2], [1, kw]],
                    )

                # merged-across-images variants: fold (plane, x) into one
                # contiguous dim (kw == TBW when X0 == 0) -> 4-level APs
                def cab_all(row0):
                    return _mk_ap(
                        tb[:], row0 * 2 * TBW,
                        [[G * 6 * 2 * TBW, 128], [6 * 2 * TBW, G],
                         [2 * 2 * TBW, 2], [1, 2 * TBW]],
                    )

                def c_all(gg):
                    return _mk_ap(
                        tb[:], 3 * gg * 2 * TBW,
                        [[G * 6 * 2 * TBW, 128], [6 * 2 * TBW, G],
                         [2 * 2 * TBW, 2], [1, 2 * TBW]],
                    )

                def pv_all(tile):
                    return _mk_ap(
                        tile[:], 0,
                        [[G * 2 * 2 * kw, 128], [2 * 2 * kw, G],
                         [2 * kw, 2], [1, 2 * kw]],
                    )

                def s_all(tile, gg):
                    return _mk_ap(
                        tile[:], gg * 2 * kw,
                        [[G * R * 2 * kw, 128], [R * 2 * kw, G],
                         [2 * 2 * kw, 2], [1, 2 * kw]],
                    )

                if blk == 0 or kw != TBW:
                    # per-image: starts right after image 0's cast
                    for g in range(G):
                        ca = tb[:, g, 1:5:2, :, K0 : K0 + kw]
                        cb = tb[:, g, 2:6:2, :, K0 : K0 + kw]
                        TT(pvmax[:, g], ca, cb, Alu.max)
                        TT(pvmin[:, g], ca, cb, Alu.min)
                        for gg in range(2):
                            TT(s_ap(s2, g, gg), pv_ap(pvmax, g), c_ap(g, gg), Alu.max)
                            TT(s_ap(s0, g, gg), pv_ap(pvmin, g), c_ap(g, gg), Alu.min)
                            TT(s_ap(s1, g, gg), pv_ap(pvmax, g), c_ap(g, gg), Alu.min)
                            TT(s_ap(s1, g, gg), pv_ap(pvmin, g), s_ap(s1, g, gg), Alu.max)
                else:
                    # both images per instruction: half the instruction count
                    TT(pv_all(pvmax), cab_all(1), cab_all(2), Alu.max)
                    TT(pv_all(pvmin), cab_all(1), cab_all(2), Alu.min)
                    for gg in range(2):
                        TT(s_all(s2, gg), pv_all(pvmax), c_all(gg), Alu.max)
                        TT(s_all(s0, gg), pv_all(pvmin), c_all(gg), Alu.min)
                        TT(s_all(s1, gg), pv_all(pvmax), c_all(gg), Alu.min)
                        TT(s_all(s1, gg), pv_all(pvmin), s_all(s1, gg), Alu.max)
                return s0, s1, s2

            def phase2(E, mid, tag, s0, s1, s2, yv, img0, X0, OW, blk):
                """Horizontal merge in plane layout + merged per-image store."""
                TT = lambda out, in0, in1, op: E.tensor_tensor(
                    out=out, in0=in0, in1=in1, op=op
                )
                K0, KW = X0 // 2, OW // 2

                def EP(s):   # E plane, pair cols   k
                    return s[:, :, :, 0, 1 : 1 + KW]
                def OP(s):   # O plane, pair cols   k
                    return s[:, :, :, 1, 1 : 1 + KW]
                def OL(s):   # O plane, cols  k-1
                    return s[:, :, :, 1, 0:KW]
                def ER(s):   # E plane, cols  k+1
                    return s[:, :, :, 0, 2 : 2 + KW]

                pM = mid.tile([128, G, R, KW], bf16, tag=f"pM{tag}")
                A = mid.tile([128, G, R, 2, KW], bf16, tag=f"A{tag}")
                pm2 = mid.tile([128, G, R, KW], bf16, tag=f"pm2{tag}")
                Cc = mid.tile([128, G, R, 2, KW], bf16, tag=f"Cc{tag}")
                # A = max3(s0) per plane
                TT(pM[:], EP(s0), OP(s0), Alu.max)
                TT(A[:, :, :, 0, :], pM[:], OL(s0), Alu.max)
                TT(A[:, :, :, 1, :], pM[:], ER(s0), Alu.max)
                # Cc = min3(s2) per plane
                TT(pm2[:], EP(s2), OP(s2), Alu.min)
                TT(Cc[:, :, :, 0, :], pm2[:], OL(s2), Alu.min)
                TT(Cc[:, :, :, 1, :], pm2[:], ER(s2), Alu.min)
                # B = med3(s1) per plane
                pbm = mid.tile([128, G, R, KW], bf16, tag=f"pbm{tag}")
                pbM = mid.tile([128, G, R, KW], bf16, tag=f"pbM{tag}")
                tB = mid.tile([128, G, R, 2, KW], bf16, tag=f"tB{tag}")
                TT(pbm[:], EP(s1), OP(s1), Alu.min)
                TT(pbM[:], EP(s1), OP(s1), Alu.max)
                TT(tB[:, :, :, 0, :], pbM[:], OL(s1), Alu.min)
                TT(tB[:, :, :, 0, :], pbm[:], tB[:, :, :, 0, :], Alu.max)
                TT(tB[:, :, :, 1, :], pbM[:], ER(s1), Alu.min)
                TT(tB[:, :, :, 1, :], pbm[:], tB[:, :, :, 1, :], Alu.max)
                # final med3(A, B, Cc) per image g (planes folded, contiguous)
                u = mid.tile([128, G, R, 2, KW], bf16, tag=f"u{tag}")
                out_t = outp.tile([128, G, R, 2, KW], bf16, tag=f"out{tag}")
                if blk < NBLK - 1:
                    # whole-block finals (planes fold contiguously)
                    TT(u[:], A[:], tB[:], Alu.min)
                    TT(A[:], A[:], tB[:], Alu.max)
                    TT(Cc[:], A[:], Cc[:], Alu.min)
                    TT(out_t[:], u[:], Cc[:], Alu.max)
                    with tc.high_priority(offset=70):
                        for g in range(G):
                            yr = yv[img0 + g].rearrange("(p r) w -> p r w", p=128)
                            nc.sync.dma_start(
                                out=yr[:, :, 2 * K0 : 2 * (K0 + KW)],
                                in_=out_t[:, g],
                            )
                else:
                    # last block: fine-grained so stores overlap the tail
                    for g in range(G):
                        for pl in range(2):
                            TT(u[:, g, :, pl], A[:, g, :, pl], tB[:, g, :, pl], Alu.min)
                            TT(A[:, g, :, pl], A[:, g, :, pl], tB[:, g, :, pl], Alu.max)
                            TT(Cc[:, g, :, pl], A[:, g, :, pl], Cc[:, g, :, pl], Alu.min)
                            TT(out_t[:, g, :, pl], u[:, g, :, pl], Cc[:, g, :, pl], Alu.max)
                            with tc.high_priority(offset=70):
                                yr = yv[img0 + g].rearrange("(p r) w -> p r w", p=128)
                                nc.sync.dma_start(
                                    out=yr[:, :, 2 * K0 + pl * KW : 2 * K0 + (pl + 1) * KW],
                                    in_=out_t[:, g, :, pl],
                                )

            # tiny ACT warmup so LoadActFuncSet runs at t=0, off the
            # critical path of the first cast
            warm = castp.tile([128, 2], bf16, tag="warm")
            nc.scalar.memzero(warm[:])

            for _rep in range(int(_os.environ.get("KREPS", "1"))):
              for blk in range(NBLK):
                img0 = blk * G

                # ---- load f32 central rows (per image):
                # t[p, g, r, 1+w] = x[img0+g, 4p+r, w]
                t = inp.tile([128, G, R, WP], f32, tag="t_in")
                for g in range(G):
                    xr = x[img0 + g].rearrange("(p r) w -> p r w", p=128)
                    nc.sync.dma_start(out=t[:, g, 0:2, 1 : W + 1], in_=xr[:, 0:2, :])
                    nc.sync.dma_start(out=t[:, g, 2:4, 1 : W + 1], in_=xr[:, 2:4, :])

                # ---- bf16 plane tile with halo rows:
                # tb[p, g, j, pl, 1+k] = x[img0+g, 4p+j-1, 2k+pl], j=1..4
                tb = castp.tile([128, G, 6, 2, TBW], bf16, tag="tb")
                # zero halo rows 0 and 5 (all partitions, both planes) (Pool,
                # keeping ACT free to start casting as soon as loads land)
                nc.gpsimd.memset(tb[:, :, 0:6:5, :, :], 0.0)
                for g in range(G):
                    # zero pad cols 0,257 of rows 1..4 (tiny, Pool)
                    nc.gpsimd.memset(tb[:, g, 1:5, :, 0 : TBW : TBW - 1], 0.0)
                    # cast central rows of image g into planes (ACT)
                    for rh in range(2):
                        nc.scalar.copy(
                            out=tb[:, g, 1 + 2 * rh : 3 + 2 * rh, 0, 1 : 1 + WH],
                            in_=t[:, g, 2 * rh : 2 * rh + 2, 1 : W + 1 : 2],
                        )
                        nc.scalar.copy(
                            out=tb[:, g, 1 + 2 * rh : 3 + 2 * rh, 1, 1 : 1 + WH],
                            in_=t[:, g, 2 * rh : 2 * rh + 2, 2 : W + 2 : 2],
                        )
                    # halo above: tb row 0 of partition p = tb row 4 of p-1
                    nc.sync.dma_start(
                        out=tb[1:128, g, 0, :, :], in_=tb[0:127, g, 4, :, :]
                    )
                    # halo below: tb row 5 of partition p = tb row 1 of p+1
                    nc.sync.dma_start(
                        out=tb[0:127, g, 5, :, :], in_=tb[1:128, g, 1, :, :]
                    )

                # ---- the network, all on DVE (neuronxcc rejects Pool min/max)
                sv = phase1(nc.vector, midv, "v", tb, 0, XSPLIT, blk)
                phase2(nc.vector, midv, "v", *sv, y, img0, 0, XSPLIT, blk)
    nc.compile()
    return nc


def _get_nc():
    if "nc" not in _STATE:
        _STATE["nc"] = _build_nc()
    return _STATE["nc"]


def _decode(yarr):
    """Undo the plane-blocked store layout -> real column order."""
    KV = XSPLIT // 2
    KP = (W - XSPLIT) // 2
    out = np.empty_like(yarr)
    out[..., 0:XSPLIT:2] = yarr[..., 0:KV]
    out[..., 1:XSPLIT:2] = yarr[..., KV : 2 * KV]
    out[..., XSPLIT::2] = yarr[..., 2 * KV : 2 * KV + KP]
    out[..., XSPLIT + 1 :: 2] = yarr[..., 2 * KV + KP :]
    return out


def kernel(x: np.ndarray) -> np.ndarray:
    from concourse.bass_utils import run_bass_kernel_spmd

    x = np.ascontiguousarray(np.asarray(x, dtype=np.float32))
    assert x.shape == (B, C, H, W), x.shape

    nc = _get_nc()
    in_maps = [
        {"x": x[i * B_LOC : (i + 1) * B_LOC].reshape(IMGS, H, W)}
        for i in range(N_CORES)
    ]
    res = run_bass_kernel_spmd(nc, in_maps, core_ids=list(range(N_CORES)))
    _STATE["last_results"] = res
    out = np.empty((B, C, H, W), dtype=np.float32)
    ov = out.reshape(N_CORES, IMGS, H, W)
    for i, r in enumerate(res.results):
        ov[i] = _decode(np.asarray(r["y"]).astype(np.float32))
    return out
